# revision 27
# baseline (speedup 1.0000x reference)
"""Trainium2 Bass kernel for dual-input multi-head attention.

Computes, for each of two independent inputs x, y of shape [8, 1024, 768]:
    qkv = inp @ w_qkv.T ; split into 12 heads of 64
    attn = softmax(q k^T / sqrt(64)) v
    out  = attn @ w_proj.T + b_proj
Sharded data-parallel over the batch dim: core i handles batch i of x AND
batch i of y (16 batch-units over 8 cores = 2 per core).

Per-core design (successor of the ~338us emit_attn_pair version; this one
restructures emission into one globally software-pipelined slot stream):
  - Host pre-transposes to bf16 chunk-major layouts ([128, CT, cols]) so
    each input loads with a handful of large DMA descriptors instead of
    ~24 per-chunk ones (descriptor issue on the Sync queue costs ~600ns
    each and was serializing input arrival).  Descriptors are ordered
    startup-critical-first: wq jt0, wq jt6, x first-half, then the rest.
  - ~50 dummy warm-up matmuls (un-throttle the PE HAM clock gate to
    2.4 GHz) sized to end right as the leading DMAs land.
  - Attention is a flat sequence of 192 slots (idx, g-half, head-pair,
    key-tile).  Scores are row-packed (two heads at PE row groups
    0-63/64-127, concurrent); exp on ScalarE straight out of PSUM
    ([128,1024] per call, 1/8 scale folded in; scores are O(+-15) so no
    max subtraction); P@V carries a 65th all-ones v-column so the softmax
    denominator falls out as PSUM row 64 for free.
  - Cross-boundary software pipelining: at iteration i the emitter issues
    scores+exp for slot i+2 and P@V for slot i-2.  The 2-slot scores
    lead means the first exp of pass p+1 is already queued before pass
    p's tail P@V, so ScalarE never idles at pass boundaries; the 2-slot
    P@V lag gives the normalization chain (DVE reciprocal + GpSimd
    broadcast) a full ~2.3us to free the P@V PSUM banks before pass
    p+1's first P@V group needs them.
  - All other matmul work (QKV emission, v tiles, both projections) lives
    in a filler queue drained a few matmuls per slot, ordered by first
    use with per-(j-tile, n-half) granularity; a forcing pass 2 slots
    ahead guarantees score/PV dependencies are in the (in-order) PE queue
    before the instructions that need them.  Fillers always sit in front
    of dependent instructions so the PE queue head never blocks while
    independent work exists.
  - Normalization is fused: the denominator row is staged to partition 0
    (custom-DVE ops misread PSUM at partition base 64), inverted with
    reciprocal_approx_fast, broadcast on the idle GpSimd engine, and
    applied by one tensor_tensor multiply that writes normalized bf16
    attnT straight from P@V PSUM.
  - proj(x) and proj(y) tokens 0-511 drain as fillers once their attnT
    halves exist; only proj(y) token tiles 4-7 remain as the epilogue.
  - PSUM budget (8 banks): 2x score tiles (4) + 2x P@V (2) + filler (2).
    Prologue/epilogue matmul groups borrow the attention tags' banks.
"""

from collections import deque

import numpy as np

import concourse.bacc as bacc
import concourse.mybir as mybir
import concourse.tile as tile
from concourse import bass_utils

B, N, C, H, HD = 8, 1024, 768, 12, 64
NT = N // 128  # 8 token tiles
CT = C // 128  # 6 contraction chunks
SCALE = HD ** -0.5
F32 = mybir.dt.float32
BF16 = mybir.dt.bfloat16
AF = mybir.ActivationFunctionType
ALU = mybir.AluOpType
N_CORES = 8

# Reordered w_qkvT column layout (host side builds this):
#   piece A (cols 0:1024):  jt0 | jt1 | jt6 | jt7 | v-cols 0:512
#   piece B (cols 1024:2304): jt2 | jt3 | jt8 | jt9 | jt4 | jt5 | jt10 | jt11 | v-cols 512:768
JT_OFF = {0: 0, 1: 128, 6: 256, 7: 384,
          2: 1024, 3: 1152, 8: 1280, 9: 1408,
          4: 1536, 5: 1664, 10: 1792, 11: 1920}
V_OFF = {0: 512, 1: 2048}  # v group g -> SBUF/DRAM column offset
TAG_BUFS = {"mm": 2, "pv": 2, "sc": 2}
NWARM = 32


def build_program():
    nc = bacc.Bacc("TRN2", target_bir_lowering=False, debug=False)
    # chunk-major DRAM layouts: [p, c, cols] = original[c*128+p, cols]
    inp_dram = [
        nc.dram_tensor("xT", [128, CT, N], BF16, kind="ExternalInput"),
        nc.dram_tensor("yT", [128, CT, N], BF16, kind="ExternalInput"),
    ]
    wqT = nc.dram_tensor("wqT", [128, CT, 3 * C], BF16, kind="ExternalInput")
    wpT = nc.dram_tensor("wpT", [128, CT, C], BF16, kind="ExternalInput")
    bp = nc.dram_tensor("bp", [1, C], F32, kind="ExternalInput")
    out_dram = [
        nc.dram_tensor("out_x", [N, C], F32, kind="ExternalOutput"),
        nc.dram_tensor("out_y", [N, C], F32, kind="ExternalOutput"),
    ]

    with tile.TileContext(nc) as tc:
        with (
            tc.tile_pool(name="pers", bufs=1) as pers,
            tc.tile_pool(name="dbl", bufs=2) as dbl,
            tc.tile_pool(name="pexp", bufs=6) as pep,
            tc.tile_pool(name="small", bufs=2) as smp,
            tc.tile_pool(name="rbsb", bufs=3) as rbsbp,
            tc.tile_pool(name="outp", bufs=2) as outp,
            tc.tile_pool(name="ps", bufs=1, space="PSUM") as ps,
        ):
            # PE warm-up: dummy matmuls on a memset tile while input DMAs run,
            # so HAM un-throttles (1.2 -> 2.4 GHz) before real work arrives.
            wu = pers.tile([128, 512], BF16, name="wu")
            nc.vector.memset(wu[:], 0.125)
            wu_ps = ps.tile([128, 512], F32, name="wu_ps", tag="mm", bufs=2)
            for _ in range(NWARM):
                nc.tensor.matmul(wu_ps[:], wu[:, 0:128], wu[:], start=True, stop=True)

            wq_sb = pers.tile([128, CT, 3 * C], BF16, name="wq_sb")
            inp_sb = {
                0: dbl.tile([128, CT, N], BF16, name="inp_sb", tag="inp"),
                1: dbl.tile([128, CT, N], BF16, name="inp_sb2", tag="inp"),
            }
            wp_sb = pers.tile([128, CT, C], BF16, name="wp_sb")
            # startup-critical-first descriptor order; each is one large
            # descriptor so the Sync engine's ~600ns/descriptor issue rate
            # never throttles input arrival.
            nc.sync.dma_start(wq_sb[:, :, 0:128], wqT[:, :, 0:128])      # jt0
            nc.sync.dma_start(inp_sb[0][:, 0:3, 0:512], inp_dram[0][:, 0:3, 0:512])
            nc.sync.dma_start(wq_sb[:, :, 256:384], wqT[:, :, 256:384])  # jt6
            nc.sync.dma_start(inp_sb[0][:, 3:6, 0:512], inp_dram[0][:, 3:6, 0:512])
            nc.sync.dma_start(inp_sb[0][:, :, 512:1024], inp_dram[0][:, :, 512:1024])
            nc.sync.dma_start(wq_sb[:, :, 384:1024], wqT[:, :, 384:1024])  # jt7 + v grp 0
            nc.sync.dma_start(wq_sb[:, :, 128:256], wqT[:, :, 128:256])  # jt1
            nc.sync.dma_start(wq_sb[:, :, 1024:3 * C], wqT[:, :, 1024:3 * C])
            nc.sync.dma_start(inp_sb[1][:, :, :], inp_dram[1][:, :, :])
            nc.sync.dma_start(wp_sb[:, :, :], wpT[:, :, :])
            b_row = pers.tile([1, C], F32, name="b_row")
            nc.sync.dma_start(b_row[:], bp[:, :])
            bias_sb = pers.tile([128, C], F32, name="bias_sb")
            nc.gpsimd.partition_broadcast(bias_sb[:], b_row[:1, :])

            qkT_sb, v_sb, attnT_sb = {}, {}, {}
            for idx in range(2):
                # q,k transposed: j-tiles 0..5 = q (2 heads/tile), 6..11 = k
                qkT_sb[idx] = dbl.tile([128, H, N], BF16, name="qkT_sb", tag="qkT")
                # v per (token-tile, head): 64 cols of v then one col of ones
                v_sb[idx] = dbl.tile([128, NT, H, HD + 1], BF16, name="v_sb", tag="v")
                nc.vector.memset(v_sb[idx][:, :, :, HD : HD + 1], 1.0)
                # attention output, transposed [C, N] as 6 chunks of 128
                attnT_sb[idx] = dbl.tile([128, CT, N], BF16, name="attnT_sb", tag="attnT")

            # prologue/epilogue matmul groups rotate through the attention
            # tags' PSUM banks (idle then); steady-state fillers use "mm".
            PSUM_CYCLE = ("mm", "pv", "mm", "pv")
            _tag_n = [0]

            def next_tag(borrow):
                if not borrow:
                    return "mm"
                t = PSUM_CYCLE[_tag_n[0] % len(PSUM_CYCLE)]
                _tag_n[0] += 1
                return t

            def gen_qkT(idx, jt, g, copy_engine, borrow=False):
                # qkvT[j, n] = sum_c w_qkvT[c, j] inpT[c, n]; one g-half
                off = JT_OFF[jt]
                tg = next_tag(borrow)
                ps_qk = ps.tile([128, 512], F32, name="ps_qk", tag=tg, bufs=TAG_BUFS[tg])
                for c in range(CT):
                    nc.tensor.matmul(
                        ps_qk[:],
                        wq_sb[:, c, off : off + 128],
                        inp_sb[idx][:, c, g * 512 : (g + 1) * 512],
                        start=(c == 0),
                        stop=(c == CT - 1),
                    )
                    yield
                dst = qkT_sb[idx][:, jt, g * 512 : (g + 1) * 512]
                if copy_engine == "act":
                    nc.scalar.copy(dst, ps_qk[:])
                else:
                    nc.vector.tensor_copy(dst, ps_qk[:])

            def emit_qkT(idx, jt, g, copy_engine, borrow=False):
                for _ in gen_qkT(idx, jt, g, copy_engine, borrow):
                    pass

            def gen_v(idx, nt, g, copy_engine, borrow=False):
                # v[n, j] = sum_c inpT[c, n] w_qkvT[c, 2C + j]
                w = 512 if g == 0 else 256
                off = V_OFF[g]
                tg = next_tag(borrow)
                ps_v = ps.tile([128, 512], F32, name="ps_v", tag=tg, bufs=TAG_BUFS[tg])
                for c in range(CT):
                    nc.tensor.matmul(
                        ps_v[:, :w],
                        inp_sb[idx][:, c, nt * 128 : (nt + 1) * 128],
                        wq_sb[:, c, off : off + w],
                        start=(c == 0),
                        stop=(c == CT - 1),
                    )
                    yield
                hview = ps_v[:, :w].rearrange("p (h d) -> p h d", d=HD)
                dst = v_sb[idx][:, nt, g * 8 : g * 8 + w // HD, 0:HD]
                if copy_engine == "act":
                    nc.scalar.copy(dst, hview)
                else:
                    nc.vector.tensor_copy(dst, hview)

            def gen_proj(idx, nt, borrow=False):
                # p1's bias-add is emitted BEFORE p2 is allocated: with the
                # shared "mm" tag, p2's slot reuse needs p1's reader already
                # in the schedule or the WAR dependency deadlocks.
                out_sb = outp.tile([128, C], F32, name="out_sb", tag="outsb")
                p1 = ps.tile([128, 512], F32, name="p1", tag=(t1 := next_tag(borrow)), bufs=TAG_BUFS[t1])
                for c in range(CT):
                    nc.tensor.matmul(
                        p1[:],
                        attnT_sb[idx][:, c, nt * 128 : (nt + 1) * 128],
                        wp_sb[:, c, 0:512],
                        start=(c == 0),
                        stop=(c == CT - 1),
                    )
                    yield
                nc.vector.tensor_tensor(
                    out_sb[:, 0:512], p1[:], bias_sb[:, 0:512], op=ALU.add
                )
                p2 = ps.tile([128, 512], F32, name="p2", tag=(t2 := next_tag(borrow)), bufs=TAG_BUFS[t2])
                for c in range(CT):
                    nc.tensor.matmul(
                        p2[:, :256],
                        attnT_sb[idx][:, c, nt * 128 : (nt + 1) * 128],
                        wp_sb[:, c, 512:768],
                        start=(c == 0),
                        stop=(c == CT - 1),
                    )
                    yield
                nc.vector.tensor_tensor(
                    out_sb[:, 512:768], p2[:, :256], bias_sb[:, 512:768], op=ALU.add
                )
                nc.sync.dma_start(out_dram[idx][nt * 128 : (nt + 1) * 128, :], out_sb[:])

            def emit_proj(idx, nt, borrow=False):
                for _ in gen_proj(idx, nt, borrow):
                    pass

            fillers = deque()  # (generator, tag) yielding once per PE matmul
            done_tags = set()
            drained_mms = [0]

            def _step():
                # advance one filler matmul; returns False when no work left
                while fillers:
                    try:
                        next(fillers[0][0])
                        drained_mms[0] += 1
                        return True
                    except StopIteration:
                        done_tags.add(fillers[0][1])
                        fillers.popleft()
                return False

            def drain_mm(k):
                while k > 0 and _step():
                    k -= 1

            def force(tag):
                # emit filler work until the generator tagged `tag` finishes
                while tag not in done_tags and _step():
                    pass

            def drain_all():
                while _step():
                    pass

            ones1 = pers.tile([1, HD], BF16, name="ones1")
            nc.vector.memset(ones1[:], 1.0)

            def emit_norm(idx, t, g, pvs, last=False):
                # per head: fast-reciprocal of the ones-column denominator row
                # (row HD of the P@V PSUM), GpSimd-broadcast to [64,512], and
                # multiply P@V PSUM straight into the bf16 transposed
                # attention buffer (fused copy+normalize).
                gs = slice(g * 512, (g + 1) * 512)
                for ab, pv in enumerate(pvs):
                    pb = ab * 64
                    # custom-DVE recip mishandles PSUM reads at partition
                    # base 64 on HW: stage the denominator row to partition 0
                    # in SBUF first with a plain copy.
                    dn = smp.tile([1, 512], F32, name="dn", tag=f"dn{ab}")
                    nc.vector.tensor_copy(dn[0:1, :], pv[HD : HD + 1, :])
                    rc = smp.tile([1, 512], F32, name="rc", tag=f"rc{ab}")
                    nc.vector.reciprocal_approx_fast(rc[0:1, :], dn[0:1, :])
                    rb = rbsbp.tile([64, 512], F32, name="rb", tag="rb")
                    nc.gpsimd.partition_broadcast(rb[:], rc[0:1, :])
                    nc.vector.tensor_tensor(
                        attnT_sb[idx][pb : pb + 64, t, gs],
                        pv[0:HD, :],
                        rb[:],
                        op=ALU.mult,
                    )

            def sc_exp(slot):
                # 2 heads per call: scores row-packed (two heads in PE row
                # groups 0-63/64-127, adjacent emission -> concurrent), then
                # one ScalarE exp over the whole [128, 1024] PSUM tile.
                idx, g, t, mt = slot
                sc = ps.tile([128, 2, 512], F32, name="sc", tag="sc", bufs=2)
                for ab in range(2):
                    pb = ab * 64
                    nc.tensor.matmul(
                        sc[:, ab, :],
                        qkT_sb[idx][pb : pb + 64, 6 + t, mt * 128 : (mt + 1) * 128],
                        qkT_sb[idx][pb : pb + 64, t, g * 512 : (g + 1) * 512],
                        start=True,
                        stop=True,
                        tile_position=(pb, 0),
                    )
                pe = pep.tile([128, N], BF16, name="pe", tag="pexp")
                nc.scalar.activation(
                    pe[:],
                    sc[:].rearrange("p a b -> p (a b)"),
                    AF.Exp,
                    scale=SCALE,
                )
                return pe

            # ---- flat slot stream with cross-boundary pipelining ----
            slots = [
                (idx, g, t, mt)
                for idx in range(2)
                for g in range(2)
                for t in range(H // 2)
                for mt in range(NT)
            ]
            NSLOT = len(slots)

            def sc_deps(slot):
                idx, g, t, mt = slot
                return (f"qk{idx}_{t}g{g}", f"qk{idx}_{6 + t}g{mt // 4}")

            def pv_dep(slot):
                idx, g, t, mt = slot
                return f"v{idx}_{0 if t < 4 else 1}_{mt}"

            # prologue: only what the first two score tiles need
            emit_qkT(0, 0, 0, "act", borrow=True)
            emit_qkT(0, 6, 0, "act", borrow=True)
            done_tags.update({"qk0_0g0", "qk0_6g0"})

            # fillers in first-use order
            def add_qk(idx, jt, g):
                fillers.append((gen_qkT(idx, jt, g, "dve"), f"qk{idx}_{jt}g{g}"))

            def add_v(idx, nt, g):
                fillers.append((gen_v(idx, nt, g, "dve"), f"v{idx}_{g}_{nt}"))

            for idx in range(2):
                if idx == 1:
                    add_qk(1, 0, 0)
                    add_qk(1, 6, 0)
                add_qk(idx, 6, 1)
                for nt in range(NT):
                    add_v(idx, nt, 0)
                for t in range(1, 4):
                    add_qk(idx, t, 0)
                    add_qk(idx, 6 + t, 0)
                    add_qk(idx, 6 + t, 1)
                add_qk(idx, 4, 0)
                add_qk(idx, 10, 0)
                add_qk(idx, 10, 1)
                for nt in range(NT):
                    add_v(idx, nt, 1)
                add_qk(idx, 5, 0)
                add_qk(idx, 11, 0)
                add_qk(idx, 11, 1)
                for t in range(H // 2):
                    add_qk(idx, t, 1)

            # total filler matmuls: qkT gens (6 each), v gens (6), proj (12)
            TOTAL_FILLER_MMS = 46 * 6 + 32 * 6 + 12 * 12
            PV_LAG = 4  # slots between exp emission and its P@V consumption
            pe_t = {}
            pvs_of_pass = {}
            pe_t[0] = sc_exp(slots[0])
            pe_t[1] = sc_exp(slots[1])
            for i in range(NSLOT + PV_LAG):
                if i + 2 < NSLOT:
                    for tag in sc_deps(slots[i + 2]):
                        force(tag)
                # force v tiles PV_LAG slots before their P@V needs them so
                # the 6-matmul v-drain doesn't land in the dependent slot
                if i < NSLOT:
                    force(pv_dep(slots[i]))
                # paced drain: spread remaining filler work evenly over the
                # remaining slots so the tail never runs dry (exp-bound).
                left = TOTAL_FILLER_MMS - drained_mms[0]
                rem = NSLOT + PV_LAG - i
                drain_mm(max(2, min(4, -(-left // rem))))
                if PV_LAG <= i:
                    idx, g, t, mt = slots[i - PV_LAG]
                    if mt == 0:
                        pvs_of_pass[(idx, g, t)] = [
                            ps.tile([HD + 1, 512], F32, name="pv", tag="pv", bufs=2)
                            for _ in range(2)
                        ]
                    pvs = pvs_of_pass[(idx, g, t)]
                    pe = pe_t.pop(i - PV_LAG)
                    for ab in range(2):
                        nc.tensor.matmul(
                            pvs[ab][:, :],
                            v_sb[idx][:, mt, 2 * t + ab, :],
                            pe[:, ab * 512 : (ab + 1) * 512],
                            start=(mt == 0),
                            stop=(mt == NT - 1),
                        )
                    if mt == NT - 1:
                        emit_norm(
                            idx, t, g, pvs_of_pass.pop((idx, g, t)),
                            last=(i - PV_LAG == NSLOT - 1),
                        )
                if i + 2 < NSLOT:
                    pe_t[i + 2] = sc_exp(slots[i + 2])
                # proj fillers become legal once the attnT halves they read
                # are fully written (all norms of the producing pass-group).
                if i == NSLOT // 2 + PV_LAG:
                    for nt in range(NT):
                        fillers.append((gen_proj(0, nt), f"pj0_{nt}"))
                if i == NSLOT * 3 // 4 + PV_LAG:
                    for nt in range(4):
                        fillers.append((gen_proj(1, nt), f"pj1_{nt}"))

            drain_all()
            # epilogue: proj(y) token tiles 4-7.  Their c=5 chunk matmuls
            # read attnT written by the very last norm; defer those to the
            # end of each wave so the other 20 matmuls stream unblocked
            # while the norm chain completes.
            for wave in ((4, 5), (6, 7)):
                groups = []
                for nt in wave:
                    out_sb = outp.tile([128, C], F32, name="out_sb", tag="outsb")
                    p1 = ps.tile([128, 512], F32, name="p1", tag=(t1 := next_tag(True)), bufs=TAG_BUFS[t1])
                    p2 = ps.tile([128, 512], F32, name="p2", tag=(t2 := next_tag(True)), bufs=TAG_BUFS[t2])
                    groups.append((nt, out_sb, p1, p2))
                    for c in range(CT - 1):
                        nc.tensor.matmul(
                            p1[:],
                            attnT_sb[1][:, c, nt * 128 : (nt + 1) * 128],
                            wp_sb[:, c, 0:512],
                            start=(c == 0),
                            stop=False,
                        )
                    for c in range(CT - 1):
                        nc.tensor.matmul(
                            p2[:, :256],
                            attnT_sb[1][:, c, nt * 128 : (nt + 1) * 128],
                            wp_sb[:, c, 512:768],
                            start=(c == 0),
                            stop=False,
                        )
                for nt, out_sb, p1, p2 in groups:
                    nc.tensor.matmul(
                        p1[:],
                        attnT_sb[1][:, CT - 1, nt * 128 : (nt + 1) * 128],
                        wp_sb[:, CT - 1, 0:512],
                        start=False,
                        stop=True,
                    )
                    nc.vector.tensor_tensor(
                        out_sb[:, 0:512], p1[:], bias_sb[:, 0:512], op=ALU.add
                    )
                    nc.tensor.matmul(
                        p2[:, :256],
                        attnT_sb[1][:, CT - 1, nt * 128 : (nt + 1) * 128],
                        wp_sb[:, CT - 1, 512:768],
                        start=False,
                        stop=True,
                    )
                    nc.vector.tensor_tensor(
                        out_sb[:, 512:768], p2[:, :256], bias_sb[:, 512:768], op=ALU.add
                    )
                    nc.sync.dma_start(
                        out_dram[1][nt * 128 : (nt + 1) * 128, :], out_sb[:]
                    )

    nc.compile()
    return nc


_PROGRAM = None


def _get_program():
    global _PROGRAM
    if _PROGRAM is None:
        _PROGRAM = build_program()
    return _PROGRAM


def _reorder_wq_cols(wqT):
    # wqT is [C, 3C] (w_qkv.T). Build the column order described by
    # JT_OFF/V_OFF: [jt0 jt1 jt6 jt7 | v 0:512 | jt2 jt3 jt8 jt9 jt4 jt5
    # jt10 jt11 | v 512:768].
    cols = []
    for jt in (0, 1, 6, 7):
        cols.append(wqT[:, jt * 128 : (jt + 1) * 128])
    cols.append(wqT[:, 2 * C : 2 * C + 512])
    for jt in (2, 3, 8, 9, 4, 5, 10, 11):
        cols.append(wqT[:, jt * 128 : (jt + 1) * 128])
    cols.append(wqT[:, 2 * C + 512 : 3 * C])
    return np.concatenate(cols, axis=1)


def _chunk_major(a):
    # [C, cols] -> [128, CT, cols] with [p, c, :] = a[c*128 + p, :]
    return np.ascontiguousarray(
        a.reshape(CT, 128, a.shape[1]).transpose(1, 0, 2)
    )


def make_in_maps(x, y, w_qkv, w_proj, b_proj):
    import ml_dtypes

    bf = ml_dtypes.bfloat16
    x = np.asarray(x, np.float32)
    y = np.asarray(y, np.float32)
    xT = np.ascontiguousarray(
        x.transpose(0, 2, 1).reshape(B, CT, 128, N).transpose(0, 2, 1, 3)
    ).astype(bf)
    yT = np.ascontiguousarray(
        y.transpose(0, 2, 1).reshape(B, CT, 128, N).transpose(0, 2, 1, 3)
    ).astype(bf)
    wqT = _chunk_major(_reorder_wq_cols(np.asarray(w_qkv, np.float32).T)).astype(bf)
    wpT = _chunk_major(np.ascontiguousarray(np.asarray(w_proj, np.float32).T)).astype(bf)
    bpv = np.ascontiguousarray(np.asarray(b_proj, np.float32).reshape(1, C))
    return [
        {"xT": xT[i], "yT": yT[i], "wqT": wqT, "wpT": wpT, "bp": bpv}
        for i in range(N_CORES)
    ]


def kernel(x, y, w_qkv, w_proj, b_proj):
    nc = _get_program()
    in_maps = make_in_maps(x, y, w_qkv, w_proj, b_proj)
    res = bass_utils.run_bass_kernel_spmd(nc, in_maps, core_ids=list(range(N_CORES)))
    xo = np.stack([np.asarray(res.results[i]["out_x"]) for i in range(N_CORES)])
    yo = np.stack([np.asarray(res.results[i]["out_y"]) for i in range(N_CORES)])
    return (xo, yo)
